# revision 1
# baseline (speedup 1.0000x reference)
"""Trainium2 Bass kernel for CustomApplyTimeChannel.

Computes, per (batch b, block n):
    y[b,n,:] = full_conv(x[b,n,:1096], h[b,n,:24])   # length 1119
then overlap-adds frames with hop T=1096 into out[b, :15367].

Sharding: pure data parallel over batch across 8 NeuronCores (16 b/core).

Per-core layout: 224 (n,b) rows, n-major (p = n*16 + b), split into two
partition tiles (128 + 96).  The 24 taps are split across three engines,
all fp32-exact:
  - tap 0 on ScalarE: Y[:, 0:T] = x * h[:, 0]  (activation scale is [P,1])
  - taps 1..NPE on TensorE: PSUM_Y += diag(h[:, j]) @ Xpad_shifted, where
    the diagonal weight is built on ScalarE as eye * h[:, j] and Xpad is
    zero-padded so every tap streams the full 1119 output columns
  - taps NPE+1..23 on VectorE as fused MACs:
    Y[:, j:j+T] = (x * h[:, j]) + Y[:, j:j+T]   (scalar_tensor_tensor)
then Y += PSUM_Y.  The overlap-add between frame n-1's tail and frame n's
head is a partition shift by +16, done with three small SBUF->SBUF DMAs
and two 23-wide adds; the last frame's tail is stored directly.

This container's walrus compiler accepts only ONE sync wait per
instruction; see _patch_drain_split/_audit_single_wait and the "join"
copies below for how the kernel is kept within that limit.
"""

import sys

sys.path.insert(0, "/opt/trn_rl_repo")

import numpy as np

from concourse import bass, tile
from concourse import mybir
from concourse.bass_utils import run_bass_kernel_spmd

# Problem constants (hardcoded; kernel.py must be self-contained).
B = 128          # total batch
NB = 14          # channel blocks
T = 1096         # time samples per block
L = 24           # taps
F = T + L - 1    # frame length 1119
OUT_LEN = (NB - 1) * T + F   # 15367
N_CORES = 8
BC = B // N_CORES            # 16 batches per core
ROWS = NB * BC               # 224 rows per core
P0 = 128                     # tile0 partitions (n in 0..7)
P1 = ROWS - P0               # tile1 partitions = 96 (n in 8..13)

FP32 = mybir.dt.float32

# Taps 1..NPE (per tile) run on the TensorE as diag-weight matmuls
# accumulating in PSUM; taps NPE+1..23 run on DVE as fused STT MACs.
NPE = 12
WX = 23 + T + 33  # padded x width (1152): 23 left zeros, 33 right zeros

_CACHE = {}


def _patch_drain_split():
    """The walrus build here allows ONE sync wait per instruction; Tile's
    kernel-tail drain carries one wait per outstanding processor.  Split the
    drain into a chain of single-wait drains (same position, same sems)."""
    if getattr(tile.TileContext, "_drain_split_patched", False):
        return
    from concourse.vector_clock import ScopedClock

    def _drain_and_barrier(self, tick_clock, wait_clock):
        drain_inst = self.nc.sync.drain()
        wait_clock.add_sem_waits(
            drain_inst.ins, ScopedClock({None: tick_clock.global_clock})
        )
        si = drain_inst.ins.sync_info
        if si is not None and len(si.on_wait) > 1:
            waits = list(si.on_wait)
            drain_inst.ins.sync_info = mybir.SyncInfo(
                on_wait=[waits[0]], on_update=list(si.on_update)
            )
            for w in waits[1:]:
                d2 = self.nc.sync.drain()
                d2.ins.sync_info = mybir.SyncInfo(on_wait=[w], on_update=[])
        self.nc.all_engine_barrier()
        popped = self.nc._tile_sem_poison_stack.pop()
        assert popped is self._sem_poison
        self.nc.clear_and_free_semaphores(list(self.sems.allocated().values()))
        self.nc.all_engine_barrier()

    tile.TileContext._drain_and_barrier = _drain_and_barrier
    tile.TileContext._drain_split_patched = True


_ENG_SEM_PREFIX = {
    mybir.EngineType.PE: "PE_",
    mybir.EngineType.DVE: "DVE_",
    mybir.EngineType.Activation: "Activation_",
    mybir.EngineType.Pool: "Pool_",
    mybir.EngineType.SP: "SP_",
}


def _drop_self_waits(nc):
    """An engine's instruction stream executes in order, so a wait on the
    instruction's own engine semaphore is redundant.  Drop those when an
    instruction carries more than the single wait the ISA slot allows."""
    for f in nc.m.functions:
        for blk in f.blocks:
            for ins in blk.instructions:
                si = ins.sync_info
                if si is None or len(si.on_wait) <= 1:
                    continue
                pref = _ENG_SEM_PREFIX.get(ins.engine)
                if pref is None:
                    continue
                keep = [w for w in si.on_wait if not (w.ant_name or "").startswith(pref)]
                if len(keep) < len(si.on_wait) and len(keep) <= 1:
                    ins.sync_info = mybir.SyncInfo(
                        on_wait=keep, on_update=list(si.on_update)
                    )


def _audit_single_wait(nc):
    bad = []
    for f in nc.m.functions:
        for blk in f.blocks:
            for ins in blk.instructions:
                si = ins.sync_info
                if si is not None and len(si.on_wait) > 1:
                    bad.append((type(ins).__name__, ins.name, len(si.on_wait)))
    if bad:
        raise RuntimeError(f"instructions with >1 sync wait: {bad}")


def _build_nc():
    _patch_drain_split()
    nc = bass.Bass()
    x_ext = nc.declare_dram_parameter("x", [BC, NB, T], FP32, isOutput=False)
    h_ext = nc.declare_dram_parameter("h", [BC, NB, L], FP32, isOutput=False)
    if NPE > 0:
        eye_ext = nc.declare_dram_parameter("eye", [P0, P0], FP32, isOutput=False)
    out_ext = nc.declare_dram_parameter("out", [BC, OUT_LEN], FP32, isOutput=True)

    # Row-major over (n, b): row p = n*BC + b.  Expressed as 3D (n, b, t)
    # APs on the DRAM side, matched to 3D views of the SBUF tiles.
    xv = x_ext.transpose([1, 0, 2])                    # [NB, BC, T]
    hv = h_ext.transpose([1, 0, 2])                    # [NB, BC, L]
    # Main output region: out[b, n*T + t] for t < T, as (n, b, t).
    ov = (
        out_ext[:, : NB * T]
        .rearrange("b (n t) -> b n t", n=NB, t=T)
        .transpose([1, 0, 2])
    )

    with tile.TileContext(nc) as tc:
        with (
            tc.tile_pool(name="main", bufs=1) as pool,
            tc.tile_pool(name="diag", bufs=3) as dgpool,
            tc.tile_pool(name="ps", bufs=1, space="PSUM") as pspool,
        ):
            SCR = pool.tile([64, 16], FP32, tag="scr")
            SCR2 = pool.tile([64, 16], FP32, tag="scr2")
            X0 = pool.tile([P0, WX], FP32, tag="x0")
            X1 = pool.tile([P1, WX], FP32, tag="x1")
            H0 = pool.tile([P0, L], FP32, tag="h0")
            H1 = pool.tile([P1, L], FP32, tag="h1")
            Y0 = pool.tile([P0, F], FP32, tag="y0")
            Y1 = pool.tile([P1, F], FP32, tag="y1")
            T0 = pool.tile([P0, L - 1], FP32, tag="t0")
            T1 = pool.tile([P1, L - 1], FP32, tag="t1")
            X, H, Y, TLS = [X0, X1], [H0, H1], [Y0, Y1], [T0, T1]
            PN = [P0, P1]
            if NPE > 0:
                EY = pool.tile([P0, P0], FP32, tag="eye")
                YP0 = pspool.tile([P0, F], FP32, tag="yp0")
                YP1 = pspool.tile([P1, F], FP32, tag="yp1")
                PSCR = pspool.tile([1, 8], FP32, tag="pscr")
                YP = [YP0, YP1]

            def xs(k):  # the unpadded x window
                return X[k][:, 23 : 23 + T]

            # SBUF-side APs stay 2D (Tile's dep tracking mishandles
            # partition-split views); all reordering lives on the DRAM side.
            NS = [(0, 8), (8, NB)]  # n-ranges per tile
            for k, (n0, n1) in enumerate(NS):
                nc.sync.dma_start(out=X[k][:, 23 : 23 + T], in_=xv[n0:n1])
                nc.sync.dma_start(out=H[k][:], in_=hv[n0:n1])
                # zero pads around x (DVE)
                nc.vector.memset(X[k][:, 0:23], 0.0)
                nc.vector.memset(X[k][:, 23 + T : WX], 0.0)
            if NPE > 0:
                nc.scalar.dma_start(out=EY[:], in_=eye_ext[:])

            # The engine ISA structs hold only ONE sync wait, so no compute
            # op may depend on two DMA queues at once.  Tiny "join" copies
            # absorb each DMA wait into the engine's vector clock first.
            _jc = [0]

            def join(src_tile, pb, col=0, eng=nc.vector, scr=None):
                i = _jc[0]
                _jc[0] += 1
                scr = SCR if scr is None else scr
                dst = scr[pb : pb + 1, i : i + 1]
                src = src_tile[pb : pb + 1, col : col + 1]
                if eng is nc.vector:
                    nc.vector.tensor_copy(dst, src)
                else:
                    nc.scalar.copy(dst, src)

            for k in range(2):
                join(X[k], 0, col=500)             # DVE observes x-DMA
                join(H[k], 0)                      # DVE observes h-DMA
            if NPE > 0:
                for k in range(2):
                    join(X[k], 0, col=500, eng=nc.scalar, scr=SCR2)
                    join(H[k], 0, eng=nc.scalar, scr=SCR2)
                join(EY, 0, eng=nc.scalar, scr=SCR2)

            # Tap 0 initializes Y[:, 0:T] (ScalarE); DVE memsets the tail.
            for k in range(2):
                nc.scalar.mul(Y[k][:, 0:T], xs(k), H[k][:, 0:1])
                nc.vector.memset(Y[k][:, T:F], 0.0)
            for k in range(2):
                # DVE observes the ACT tap-0 writes before the STT taps.
                join(Y[k], 0)

            if NPE > 0:
                # PE joins: dummy 1x1 matmuls absorbing the Xpad deps so the
                # real matmuls carry at most the one ACT (diag) wait.
                for i, (tl, c) in enumerate(
                    [(X0, 0), (X0, 500), (X1, 0), (X1, 500)]
                ):
                    cell = tl[0:1, c : c + 1]
                    nc.tensor.matmul(
                        PSCR[0:1, i : i + 1], cell, cell, start=True, stop=True
                    )
                # Taps 1..NPE on PE: Y_psum += diag(h_j) @ Xpad shifted views.
                PIECES = [(0, 512), (512, 1024), (1024, F)]
                for k in range(2):
                    for j in range(1, NPE + 1):
                        # distinct tile per tap: no slot-reuse WAR waits
                        DG = dgpool.tile([PN[k], PN[k]], FP32, tag=f"dg{k}_{j}")
                        nc.scalar.mul(DG[:], EY[0 : PN[k], 0 : PN[k]], H[k][:, j : j + 1])
                        for c0, c1 in PIECES:
                            nc.tensor.matmul(
                                YP[k][:, c0:c1],
                                DG[:],
                                X[k][:, 23 - j + c0 : 23 - j + c1],
                                start=(j == 1),
                                stop=(j == NPE),
                            )

            # Remaining taps on DVE as fused scalar*tensor+tensor MACs.
            for j in range(NPE + 1, L):
                for k in range(2):
                    nc.vector.scalar_tensor_tensor(
                        out=Y[k][:, j : j + T],
                        in0=xs(k),
                        scalar=H[k][:, j : j + 1],
                        in1=Y[k][:, j : j + T],
                        op0=mybir.AluOpType.mult,
                        op1=mybir.AluOpType.add,
                    )
            if NPE > 0:
                # Fold the PE partial sums into Y (DVE first observes the PE
                # clock through a one-cell PSUM read).
                for k in range(2):
                    join(YP[k], 0)
                for k in range(2):
                    nc.vector.tensor_add(Y[k][:], Y[k][:], YP[k][:])

            # POOL observes the ACT clock via a pure-ACT artifact before the
            # tail DMAs trigger, so those keep only their DVE wait (Tile's
            # DMA-side dependency tracking is tile-coarse and would otherwise
            # add a spurious ACT wait).
            SCRP = pool.tile([1, 8], FP32, tag="scrp")
            if NPE > 0:
                nc.gpsimd.tensor_copy(SCRP[0:1, 0:1], DG[0:1, 0:1])
            else:
                nc.scalar.copy(SCR2[0:1, 15:16], SCR2[0:1, 0:1])
                nc.gpsimd.tensor_copy(SCRP[0:1, 0:1], SCR2[0:1, 15:16])

            # Overlap-add: row p (= n*BC + b) with n >= 1 needs the tail of
            # row p - BC added to its head.  Shift tails down 16 partitions.
            nc.vector.memset(TLS[0][0:BC, :], 0.0)          # n = 0: no tail
            nc.gpsimd.dma_start(out=TLS[0][BC:P0, :], in_=Y[0][0 : P0 - BC, T:F])
            nc.gpsimd.dma_start(out=TLS[1][0:BC, :], in_=Y[0][P0 - BC : P0, T:F])
            nc.gpsimd.dma_start(out=TLS[1][BC:P1, :], in_=Y[1][0 : P1 - BC, T:F])
            # In-place one-cell copies on the TLS tiles: absorb each tail
            # DMA's queue semaphore into DVE AND create a write the adds
            # depend on, forcing join-before-add scheduling.
            nc.vector.tensor_copy(TLS[0][32:33, 0:1], TLS[0][32:33, 0:1])
            nc.vector.tensor_copy(TLS[1][0:1, 1:2], TLS[1][0:1, 1:2])
            nc.vector.tensor_copy(TLS[1][32:33, 1:2], TLS[1][32:33, 1:2])
            for k in range(2):
                nc.vector.tensor_add(Y[k][:, 0 : L - 1], Y[k][:, 0 : L - 1], TLS[k][:])

            # Store main frames and the final tail (last frame's spill), on
            # the POOL descriptor ring (1 wait each: the DVE completion).
            for k, (n0, n1) in enumerate(NS):
                nc.gpsimd.dma_start(out=ov[n0:n1], in_=Y[k][:, 0:T])
            nc.gpsimd.dma_start(
                out=out_ext[:, NB * T : OUT_LEN],
                in_=Y[1][P1 - BC : P1, T:F],
            )
    _audit_single_wait(nc)
    return nc


def _get_nc():
    if "nc" not in _CACHE:
        _CACHE["nc"] = _build_nc()
    return _CACHE["nc"]


def _run(x, h_time, trace=False, **kw):
    x = np.ascontiguousarray(np.asarray(x, dtype=np.float32))
    h = np.ascontiguousarray(np.asarray(h_time, dtype=np.float32))
    nc = _get_nc()
    eye = np.eye(P0, dtype=np.float32)
    in_maps = [
        {"x": x[i * BC : (i + 1) * BC], "h": h[i * BC : (i + 1) * BC]}
        for i in range(N_CORES)
    ]
    if NPE > 0:
        for m in in_maps:
            m["eye"] = eye
    res = run_bass_kernel_spmd(nc, in_maps, list(range(N_CORES)), trace=trace, **kw)
    out = np.concatenate([res.results[i]["out"] for i in range(N_CORES)], axis=0)
    return out.astype(np.float32), res


def kernel(x, h_time):
    out, _ = _run(x, h_time, trace=False)
    return out


if __name__ == "__main__":
    # Dry build: just construct the program and report instruction counts.
    nc = _build_nc()
    from collections import Counter

    cnt = Counter()
    for f in nc.m.functions:
        for blk in f.blocks:
            for ins in blk.instructions:
                cnt[type(ins).__name__] += 1
    print(dict(cnt))
    print("total instructions:", sum(cnt.values()))



# revision 3
# speedup vs baseline: 16.1202x; 16.1202x over previous
"""Trainium2 Bass kernel for CustomApplyTimeChannel.

Per (batch b, block n): y[b,n,:] = full_conv(x[b,n,:1096], h[b,n,:24]),
then overlap-add with hop T=1096 into out[b, :15367].
Pure data parallel over batch across 8 NeuronCores (16 b/core); per-core
rows p = n*16 + b (n-major), split into two partition tiles (128 + 96).

Engine split of the 24 taps (per tile):
  - taps 1..18 / 1..17 on PE as bf16 diag-weight matmuls accumulating
    fp32 in PSUM.  The ISA caps a matmul at one PSUM bank (512 fp32
    cols), so the 1119 output columns are split into three pieces with
    SEPARATE PSUM tiles and SEPARATE bf16 x-window tiles per piece —
    Tile's dependency tracking is tile-coarse, so per-piece tiles let
    each piece's fold run as soon as that piece's group retires.
  - tap 0 + the remaining taps on DVE as fp32 TensorScalarPtr MACs.
  - Constants are built on-device: tile-0 diag weights + the two
    overlap-add shift matrices on Pool via affine_select (from a
    stride-0 broadcast of h), tile-1 diag weights on ACT as eye*h.
  - ACT casts x to bf16 (three window tiles per partition tile).
The overlap-add shifts frame tails down 16 partitions with three tiny
shift-matrix matmuls into PSUM plus two narrow DVE adds.  All DMAs ride
the SP/ACT hardware DGE queues (gpsimd software DGE is far slower per
descriptor).

This walrus accepts only ONE sync wait per instruction; any multi-wait
instruction is legalized post-schedule by hoisting extra waits onto
single-wait Drain instructions on the same engine (see
_legalize_single_wait).

The host path compiles the PJRT executable once and caches it; repeat
kernel() calls only device_put the inputs and dispatch.  If BASS_TRACE
is set, execution routes through run_bass_kernel_spmd so NTFF profiling
hooks (where available) still observe the kernel.
"""

import os
import sys

sys.path.insert(0, "/opt/trn_rl_repo")

import numpy as np

from concourse import bass, tile, mybir

# Problem constants (hardcoded; kernel.py must be self-contained).
B = 128          # total batch
NB = 14          # channel blocks
T = 1096         # time samples per block
L = 24           # taps
F = T + L - 1    # frame length 1119
OUT_LEN = (NB - 1) * T + F   # 15367
N_CORES = 8
BC = B // N_CORES            # 16 batches per core
ROWS = NB * BC               # 224 rows per core
P0 = 128                     # tile0 partitions (n in 0..7)
P1 = ROWS - P0               # tile1 partitions = 96 (n in 8..13)

FP32 = mybir.dt.float32
BF16 = mybir.dt.bfloat16

# per-tile tap assignment (tap 0 initializes Y on DVE)
PE_TAPS = [list(range(1, 19)), list(range(1, 18))]
DVE_TAPS = [list(range(19, 24)), list(range(18, 24))]

# column pieces of the 1119-wide output, one PSUM bank each
PIECES = [(0, 512), (512, 1024), (1024, F)]

_CACHE = {}


def _legalize_single_wait(nc):
    """The walrus build here allows ONE sync wait per instruction.  For any
    instruction carrying N>1 waits, hoist N-1 of them onto bare Drain
    instructions on the SAME engine placed immediately before it: the engine
    sequencer executes in order, so the conjunction of waits is preserved.
    Must run after Tile's scheduler assigned sync_info and after the Tile
    instruction hook was popped (i.e. from within _drain_and_barrier)."""
    for f in nc.m.functions:
        for blk in f.blocks:
            snapshot = list(blk.instructions)
            if not any(
                i.sync_info is not None and len(i.sync_info.on_wait) > 1
                for i in snapshot
            ):
                continue
            created = set()
            new_list = []
            for ins in snapshot:
                si = ins.sync_info
                if si is not None and len(si.on_wait) > 1:
                    waits = list(si.on_wait)
                    for w in waits[:-1]:
                        d = nc.sync.drain()
                        d.ins.engine = ins.engine
                        d.ins.sync_info = mybir.SyncInfo(on_wait=[w], on_update=[])
                        created.add(id(d.ins))
                        new_list.append(d.ins)
                    ins.sync_info = mybir.SyncInfo(
                        on_wait=[waits[-1]], on_update=list(si.on_update)
                    )
                new_list.append(ins)
            # nc.sync.drain() appended the new drains at the end of the
            # current block; drop those trailing copies everywhere and
            # install the ordered list.
            for f2 in nc.m.functions:
                for blk2 in f2.blocks:
                    if blk2 is blk:
                        blk2.instructions[:] = new_list
                    else:
                        blk2.instructions[:] = [
                            i for i in blk2.instructions if id(i) not in created
                        ]


def _patch_drain_split():
    """Tile's kernel-tail drain carries one wait per outstanding processor;
    split it into a chain of single-wait drains, and legalize any other
    multi-wait instruction the same way."""
    if getattr(tile.TileContext, "_drain_split_patched", False):
        return
    from concourse.vector_clock import ScopedClock

    def _drain_and_barrier(self, tick_clock, wait_clock):
        _legalize_single_wait(self.nc)
        drain_inst = self.nc.sync.drain()
        wait_clock.add_sem_waits(
            drain_inst.ins, ScopedClock({None: tick_clock.global_clock})
        )
        si = drain_inst.ins.sync_info
        if si is not None and len(si.on_wait) > 1:
            waits = list(si.on_wait)
            drain_inst.ins.sync_info = mybir.SyncInfo(
                on_wait=[waits[0]], on_update=list(si.on_update)
            )
            for w in waits[1:]:
                d2 = self.nc.sync.drain()
                d2.ins.sync_info = mybir.SyncInfo(on_wait=[w], on_update=[])
        self.nc.all_engine_barrier()
        popped = self.nc._tile_sem_poison_stack.pop()
        assert popped is self._sem_poison
        self.nc.clear_and_free_semaphores(list(self.sems.allocated().values()))
        self.nc.all_engine_barrier()

    tile.TileContext._drain_and_barrier = _drain_and_barrier
    tile.TileContext._drain_split_patched = True


def _audit_single_wait(nc):
    bad = []
    for f in nc.m.functions:
        for blk in f.blocks:
            for ins in blk.instructions:
                si = ins.sync_info
                if si is not None and len(si.on_wait) > 1:
                    bad.append((type(ins).__name__, ins.name, len(si.on_wait)))
    if bad:
        raise RuntimeError(f"instructions with >1 sync wait: {bad}")


def _build_nc():
    _patch_drain_split()
    nc = bass.Bass()
    x_ext = nc.declare_dram_parameter("x", [BC, NB, T], FP32, isOutput=False)
    h_ext = nc.declare_dram_parameter("h", [BC, NB, L], FP32, isOutput=False)
    out_ext = nc.declare_dram_parameter("out", [BC, OUT_LEN], FP32, isOutput=True)

    xv = x_ext.transpose([1, 0, 2])                    # [NB, BC, T]
    hv = h_ext.transpose([1, 0, 2])                    # [NB, BC, L]
    ov = (
        out_ext[:, : NB * T]
        .rearrange("b (n t) -> b n t", n=NB, t=T)
        .transpose([1, 0, 2])
    )

    with tile.TileContext(nc) as tc:
        with (
            tc.tile_pool(name="main", bufs=1) as pool,
            tc.tile_pool(name="ps", bufs=1, space="PSUM") as pspool,
        ):
            ONES = pool.tile([P0, P0], FP32, tag="ones")
            S16 = pool.tile([P0, P0], FP32, tag="s16")
            SB = pool.tile([P0, P0], FP32, tag="sb")
            EY = pool.tile([P0, P0], FP32, tag="ey")
            X0 = pool.tile([P0, T], FP32, tag="x0")
            X1 = pool.tile([P1, T], FP32, tag="x1")
            H0 = pool.tile([P0, L], FP32, tag="h0")
            H1 = pool.tile([P1, L], FP32, tag="h1")
            Y0 = pool.tile([P0, F], FP32, tag="y0")
            Y1 = pool.tile([P1, F], FP32, tag="y1")
            X, H, Y, PN = [X0, X1], [H0, H1], [Y0, Y1], [P0, P1]
            # per-piece bf16 x windows: piece p of tap j reads columns
            # [23-j+c0, 23-j+c1) of the padded x; with per-piece tiles the
            # local window is [23-j, 23-j+piece_w) in every piece.
            XA = [
                pool.tile([PN[k], 535], BF16, tag=f"xa{k}", name=f"xa{k}")
                for k in range(2)
            ]
            XBt = [
                pool.tile([PN[k], 535], BF16, tag=f"xb{k}", name=f"xb{k}")
                for k in range(2)
            ]
            XC = [
                pool.tile([PN[k], 118], BF16, tag=f"xc{k}", name=f"xc{k}")
                for k in range(2)
            ]
            # per-piece PSUM accumulators + tail-shift tiles: 8 banks exactly
            PA = [
                pspool.tile([PN[k], 512], FP32, tag=f"pa{k}", name=f"pa{k}")
                for k in range(2)
            ]
            PB = [
                pspool.tile([PN[k], 512], FP32, tag=f"pb{k}", name=f"pb{k}")
                for k in range(2)
            ]
            PC = [
                pspool.tile([PN[k], F - 1024], FP32, tag=f"pc{k}", name=f"pc{k}")
                for k in range(2)
            ]
            PP = [PA, PB, PC]
            TP0 = pspool.tile([P0, L - 1], FP32, tag="tp0")
            TP1 = pspool.tile([P1, L - 1], FP32, tag="tp1")

            # loads: x on SP, h on ACT (both hardware DGE)
            NS = [(0, 8), (8, NB)]
            for k, (n0, n1) in enumerate(NS):
                nc.sync.dma_start(out=X[k][:], in_=xv[n0:n1])
                nc.scalar.dma_start(out=H[k][:], in_=hv[n0:n1])

            # constants: sel(p,c) keeps in_ where base + c - p == 0
            nc.vector.memset(ONES[:], 1.0)

            def sel(out_ap, in_ap, base, width=P0, fill=0.0):
                nc.gpsimd.affine_select(
                    out=out_ap, in_=in_ap, pattern=[[1, width]],
                    compare_op=mybir.AluOpType.is_equal, fill=fill,
                    base=base, channel_multiplier=-1,
                )

            sel(S16[:], ONES[:], -16)        # S16[p,c] = 1{c == p+16}
            sel(SB[:], ONES[:], 112)         # SB[p,c]  = 1{c == p-112}
            sel(EY[:], ONES[:], 0)           # EY[p,c]  = 1{c == p}
            # diag weights DG_j = diag(h[:, j]) in bf16
            DGS = {}
            for k in range(2):
                for j in PE_TAPS[k]:
                    DG = pool.tile(
                        [PN[k], PN[k]], BF16, tag=f"dg{k}_{j}", name=f"dg{k}_{j}"
                    )
                    DGS[(k, j)] = DG
            # tile0 diags on Pool via affine_select; tile1 diags on ACT via
            # EY*h so production runs on two engines in parallel
            for j in PE_TAPS[0]:
                sel(
                    DGS[(0, j)][:],
                    H[0][:, j : j + 1].broadcast_to([P0, P0]),
                    0,
                    width=P0,
                )

            # ACT: piece-A casts first so PE can start, then tile1 diags
            for k in range(2):
                nc.vector.memset(XA[k][:, 0:23], 0.0)
                nc.vector.memset(XC[k][:, 95:118], 0.0)
            for k in range(2):
                nc.scalar.copy(XA[k][:, 23:535], X[k][:, 0:512])
            for j in PE_TAPS[1]:
                nc.scalar.mul(DGS[(1, j)][:], EY[0:P1, 0:P1], H[1][:, j : j + 1])
            for k in range(2):
                nc.scalar.copy(XBt[k][:, 0:535], X[k][:, 489:1024])
                nc.scalar.copy(XC[k][:, 0:95], X[k][:, 1001:T])

            # DVE: tap 0 initializes Y[:, 0:T]; tail columns zeroed
            for k in range(2):
                nc.vector.memset(Y[k][:, T:F], 0.0)
                nc.vector.tensor_scalar_mul(Y[k][:, 0:T], X[k][:], H[k][:, 0:1])

            # PE: piece-major bf16 diag matmuls; per-piece groups retire
            # early so the DVE folds chase the PE instead of trailing it
            XP = [XA, XBt, XC]
            for p, (c0, c1) in enumerate(PIECES):
                w = c1 - c0
                for k in range(2):
                    taps = PE_TAPS[k]
                    for i, j in enumerate(taps):
                        nc.tensor.matmul(
                            PP[p][k][:, 0:w],
                            DGS[(k, j)][:],
                            XP[p][k][:, 23 - j : 23 - j + w],
                            start=(i == 0),
                            stop=(i == len(taps) - 1),
                        )

            # DVE: fp32 MAC taps into Y
            for k in range(2):
                for j in DVE_TAPS[k]:
                    nc.vector.scalar_tensor_tensor(
                        out=Y[k][:, j : j + T],
                        in0=X[k][:],
                        scalar=H[k][:, j : j + 1],
                        in1=Y[k][:, j : j + T],
                        op0=mybir.AluOpType.mult,
                        op1=mybir.AluOpType.add,
                    )

            # fold piece C first: the frame tails live in [1024, F), and the
            # tail-shift matmuls only need those columns
            for k in range(2):
                nc.vector.tensor_add(Y[k][:, 1024:F], Y[k][:, 1024:F], PC[k][:])

            # overlap-add: shift tails down 16 partitions via matmuls with
            # S16[p, c] = 1{c == p+16}; PSUM outputs must sit at base
            # partition 0, so the shift lives in the weight.  Rows 0..15 of
            # TP0 get 0 (n = 0 has no predecessor).
            nc.tensor.matmul(TP0[:], S16[:], Y0[:, T:F], start=True, stop=True)
            nc.tensor.matmul(
                TP1[:], SB[64:P0, 0:P1], Y0[64:P0, T:F], start=True, stop=False
            )
            nc.tensor.matmul(
                TP1[:], S16[0 : P1 - 16, 0:P1], Y1[0 : P1 - 16, T:F],
                start=False, stop=True,
            )

            # remaining folds, then the head adds
            for k in range(2):
                nc.vector.tensor_add(Y[k][:, 0:512], Y[k][:, 0:512], PA[k][:])
                nc.vector.tensor_add(Y[k][:, 512:1024], Y[k][:, 512:1024], PB[k][:])
            nc.vector.tensor_add(Y0[:, 0 : L - 1], Y0[:, 0 : L - 1], TP0[:])
            nc.vector.tensor_add(Y1[:, 0 : L - 1], Y1[:, 0 : L - 1], TP1[:])

            # stores: tile0 frames on SP, tile1 frames + last tail on ACT
            nc.sync.dma_start(out=ov[0:8], in_=Y0[:, 0:T])
            nc.scalar.dma_start(out=ov[8:NB], in_=Y1[:, 0:T])
            nc.scalar.dma_start(
                out=out_ext[:, NB * T : OUT_LEN], in_=Y1[P1 - BC : P1, T:F]
            )
    _audit_single_wait(nc)
    return nc


def _get_nc():
    if "nc" not in _CACHE:
        _CACHE["nc"] = _build_nc()
    return _CACHE["nc"]


def _get_compiled():
    """Build the sharded PJRT callable once; reuse across kernel() calls."""
    if "jit" in _CACHE:
        return _CACHE["jit"]
    import jax
    from jax.sharding import Mesh, PartitionSpec
    from jax.experimental.shard_map import shard_map
    from concourse.bass2jax import (
        _bass_exec_p,
        install_neuronx_cc_hook,
        partition_id_tensor,
    )

    nc = _get_nc()
    install_neuronx_cc_hook()
    partition_name = nc.partition_id_tensor.name if nc.partition_id_tensor else None
    in_names, out_names, out_avals, zero_shapes = [], [], [], []
    for alloc in nc.m.functions[0].allocations:
        if not isinstance(alloc, mybir.MemoryLocationSet):
            continue
        name = alloc.memorylocations[0].name
        if alloc.kind == "ExternalInput":
            if name != partition_name:
                in_names.append(name)
        elif alloc.kind == "ExternalOutput":
            out_names.append(name)
            shape = tuple(alloc.tensor_shape)
            dtype = mybir.dt.np(alloc.dtype)
            out_avals.append(jax.core.ShapedArray(shape, dtype))
            zero_shapes.append((shape, dtype))
    n_params = len(in_names)
    all_in_names = list(in_names) + list(out_names)
    if partition_name is not None:
        all_in_names.append(partition_name)

    def _body(*args):
        operands = list(args)
        if partition_name is not None:
            operands.append(partition_id_tensor())
        outs = _bass_exec_p.bind(
            *operands,
            out_avals=tuple(out_avals),
            in_names=tuple(all_in_names),
            out_names=tuple(out_names),
            lowering_input_output_aliases=(),
            sim_require_finite=True,
            sim_require_nnan=True,
            nc=nc,
        )
        return tuple(outs)

    devices = jax.devices()[:N_CORES]
    mesh = Mesh(np.asarray(devices), ("core",))
    n_outs = len(out_names)
    in_specs = (PartitionSpec("core"),) * (n_params + n_outs)
    out_specs = (PartitionSpec("core"),) * n_outs
    f = jax.jit(
        shard_map(
            _body, mesh=mesh, in_specs=in_specs, out_specs=out_specs,
            check_rep=False,
        ),
        keep_unused=True,
    )
    # the kernel writes every output element, so the (non-donated) zero
    # buffers are placed on device once and reused
    zeros = [
        jax.device_put(np.zeros((N_CORES * s[0], *s[1:]), d))
        for (s, d) in zero_shapes
    ]
    _CACHE["jit"] = (f, in_names, zeros)
    return _CACHE["jit"]


def _run_traced(x, h, trace):
    """BASS_TRACE path: route through run_bass_kernel_spmd so external
    NTFF profiling hooks (where present) observe the execution."""
    from concourse.bass_utils import run_bass_kernel_spmd

    nc = _get_nc()
    in_maps = [
        {"x": x[i * BC : (i + 1) * BC], "h": h[i * BC : (i + 1) * BC]}
        for i in range(N_CORES)
    ]
    res = run_bass_kernel_spmd(nc, in_maps, list(range(N_CORES)), trace=trace)
    out = np.concatenate([res.results[i]["out"] for i in range(N_CORES)], axis=0)
    return out.astype(np.float32), res


def kernel(x, h_time):
    x = np.ascontiguousarray(np.asarray(x, dtype=np.float32))
    h = np.ascontiguousarray(np.asarray(h_time, dtype=np.float32))
    if os.environ.get("BASS_TRACE"):
        out, _ = _run_traced(x, h, True)
        return out
    import jax

    f, in_names, zeros = _get_compiled()
    arrs = {"x": x, "h": h}
    dev_in = [jax.device_put(arrs[name]) for name in in_names]
    outs = f(*dev_in, *zeros)
    return np.asarray(outs[0]).astype(np.float32)


if __name__ == "__main__":
    # Dry build: construct the program and report instruction counts.
    nc = _build_nc()
    from collections import Counter

    cnt = Counter()
    for f in nc.m.functions:
        for blk in f.blocks:
            for ins in blk.instructions:
                cnt[type(ins).__name__] += 1
    print(dict(cnt))
    print("total instructions:", sum(cnt.values()))
